# revision 20
# baseline (speedup 1.0000x reference)
"""MoChA (monotonic chunkwise attention) Bass kernel for TRN2, 8 NeuronCores.

Sharding: data-parallel over batch B=64 -> 8 cores x 8 batches. Weights
replicated. Returns (cv [64,1,512], alpha [64,2000]) like the reference.

Algebraic reformulation (exact): with qm' = (query@Wq + bq)/sqrt(A),
w = Wk @ qm', c = qm'.bk (+r), the energies are e[k] = key[k].w + c, so the
two [2000,512]x[512,512] matmuls collapse to one [2000,512]x[512,2] per
batch. c_chunk is dropped: a per-row constant shift cancels in the
softmax/moving-sum normalization of beta.

Schedule: batches are processed in two groups of 4 so the scan phase of
group 0 overlaps the key pass of group 1, and the value pass of group 0
overlaps the scan phase of group 1 -- keeps the DMA ring busy end-to-end.
"""
import sys
for _p in ("/opt/trn_rl_repo", "/opt/pypackages"):
    if _p not in sys.path:
        sys.path.append(_p)

import numpy as np
from contextlib import ExitStack, nullcontext

import concourse.bass as bass
import concourse.bacc as bacc
import concourse.tile as tile
from concourse import mybir, masks
from concourse.bass_utils import run_bass_kernel_spmd

f32 = mybir.dt.float32
f32r = mybir.dt.float32r
i32 = mybir.dt.int32
AF = mybir.ActivationFunctionType
OP = mybir.AluOpType
MS = bass.MemorySpace

NCORES = 8
B = 8               # batches per core
GB = 4              # batches per group (2 groups)
K = 2000
D = 512
A = 512
NEG = float(np.finfo(np.float32).min)
EPS = 1e-6
SCALE = float(np.sqrt(A))
NQ = 4              # quarters of K, 512 k's each (last has 464)
KL = K - 3 * 512    # 464


def _build_nc(reps=1):
    nc = bacc.Bacc("TRN2", target_bir_lowering=False, debug=False)

    key_d = nc.dram_tensor("key", [B, K, D], f32, kind="ExternalInput").ap()
    val_d = nc.dram_tensor("value", [B, K, D], f32, kind="ExternalInput").ap()
    qry_d = nc.dram_tensor("query", [B, 1, D], f32, kind="ExternalInput").ap()
    msk_d = nc.dram_tensor("mask", [B, 1, K], i32, kind="ExternalInput").ap()
    aw_d = nc.dram_tensor("aw_prev", [B, K], f32, kind="ExternalInput").ap()
    nz_d = nc.dram_tensor("noise", [B, K], f32, kind="ExternalInput").ap()
    wkm_d = nc.dram_tensor("Wk_mono", [D, A], f32, kind="ExternalInput").ap()
    bkm_d = nc.dram_tensor("bk_mono", [A], f32, kind="ExternalInput").ap()
    wqm_d = nc.dram_tensor("Wq_mono", [D, A], f32, kind="ExternalInput").ap()
    bqm_d = nc.dram_tensor("bq_mono", [A], f32, kind="ExternalInput").ap()
    r_d = nc.dram_tensor("r", [1], f32, kind="ExternalInput").ap()
    wkc_d = nc.dram_tensor("Wk_chunk", [D, A], f32, kind="ExternalInput").ap()
    bkc_d = nc.dram_tensor("bk_chunk", [A], f32, kind="ExternalInput").ap()
    wqc_d = nc.dram_tensor("Wq_chunk", [D, A], f32, kind="ExternalInput").ap()
    bqc_d = nc.dram_tensor("bq_chunk", [A], f32, kind="ExternalInput").ap()

    cv_d = nc.dram_tensor("out_cv", [B, 1, D], f32, kind="ExternalOutput").ap()
    al_d = nc.dram_tensor("out_alpha", [B, K], f32, kind="ExternalOutput").ap()

    with tile.TileContext(nc) as tc, ExitStack() as ctx:
        const = ctx.enter_context(tc.tile_pool(name="const", bufs=1))
        keyq_p = ctx.enter_context(tc.tile_pool(name="keyq", bufs=3))
        keyT_p = ctx.enter_context(tc.tile_pool(name="keyT", bufs=2))
        valr_p = ctx.enter_context(tc.tile_pool(name="valr", bufs=3))
        scan_p = ctx.enter_context(tc.tile_pool(name="scan", bufs=1))
        ste_p = ctx.enter_context(tc.tile_pool(name="ste", bufs=2))
        psT = ctx.enter_context(tc.tile_pool(name="psT", bufs=3, space=MS.PSUM))
        psE = ctx.enter_context(tc.tile_pool(name="psE", bufs=2, space=MS.PSUM))
        psC = ctx.enter_context(tc.tile_pool(name="psC", bufs=2, space=MS.PSUM))

        ident = const.tile([128, 128], f32)
        masks.make_identity(nc, ident[:])
        ones1 = const.tile([1, B], f32)
        nc.gpsimd.memset(ones1[:], 1.0)

        w2all = const.tile([128, 4, 2, B], f32r)     # [d_in_chunk, dchunk, type, b]
        c2T = const.tile([B, 2], f32)               # per-batch bias (mono incl r, -)
        cg0 = const.tile([GB, 2], f32)
        cg1 = const.tile([GB, 2], f32)
        bT0 = const.tile([128, 16, GB], f32r)
        bT1 = const.tile([128, 16, GB], f32r)
        betaT_g = [bT0, bT1]
        em0_t = const.tile([GB, K], f32)
        em1_t = const.tile([GB, K], f32)
        ec0_t = const.tile([GB, K], f32)
        ec1_t = const.tile([GB, K], f32)
        e_mono_g = [em0_t, em1_t]
        e_chunk_g = [ec0_t, ec1_t]

        # ---------------- prologue: per-batch w and c ----------------
        with tc.tile_pool(name="prol", bufs=1) as prol:
            q_sb = prol.tile([B, D], f32)
            nc.sync.dma_start(q_sb[:], qry_d.rearrange("b one d -> b (one d)"))
            r2 = prol.tile([1, 2], f32)
            nc.gpsimd.memset(r2[:], 0.0)
            nc.gpsimd.dma_start(r2[0:1, 0:1], r_d.rearrange("(one x) -> one x", one=1))
            bk2 = prol.tile([128, 4, 2], f32)
            nc.gpsimd.dma_start(bk2[:, :, 0], bkm_d.rearrange("(ai p) -> p ai", p=128))
            nc.gpsimd.dma_start(bk2[:, :, 1], bkc_d.rearrange("(ai p) -> p ai", p=128))

            qT = prol.tile([128, 4, B], f32)
            for dc in range(4):
                pt = psT.tile([128, B], f32, tag="pT", name="pT")
                nc.tensor.transpose(pt[:], q_sb[:, dc * 128:(dc + 1) * 128],
                                    ident[0:B, 0:B])
                nc.vector.tensor_copy(qT[:, dc, :], pt[:])

            pc2 = psC.tile([B, 2], f32, tag="pcv", name="pcv")
            for t, (wq_d, bq_d, wk_d) in enumerate(
                    [(wqm_d, bqm_d, wkm_d), (wqc_d, bqc_d, wkc_d)]):
                wq = keyq_p.tile([128, 4, A], f32, tag="kq", name="kq")
                nc.sync.dma_start(wq[:], wq_d.rearrange("(dc p) a -> p dc a", p=128))
                bq = prol.tile([1, A], f32, tag="bq")
                nc.sync.dma_start(bq[:], bq_d.rearrange("(one a) -> one a", one=1))

                # qm' = (q @ Wq + bq) / SCALE        [B, A]
                pqm = psT.tile([B, A], f32, tag="pT", name="pT")
                for dc in range(4):
                    nc.tensor.matmul(pqm[:], qT[:, dc, :], wq[:, dc, :],
                                     start=(dc == 0), stop=False)
                nc.tensor.matmul(pqm[:], ones1[:], bq[:], start=False, stop=True)
                qm = prol.tile([B, A], f32, tag="qm")
                nc.scalar.activation(qm[:], pqm[:], AF.Copy, scale=1.0 / SCALE)

                qmT = prol.tile([128, 4, B], f32, tag="qmT")
                for ai in range(4):
                    pt = psT.tile([128, B], f32, tag="pT", name="pT")
                    nc.tensor.transpose(pt[:], qm[:, ai * 128:(ai + 1) * 128],
                                        ident[0:B, 0:B])
                    nc.vector.tensor_copy(qmT[:, ai, :], pt[:])

                # c2T[:, t] = qm' . bk_t  (accumulated across both types)
                for ai in range(4):
                    nc.tensor.matmul(pc2[:, t:t + 1], qmT[:, ai, :],
                                     bk2[:, ai, t:t + 1],
                                     start=(ai == 0), stop=False,
                                     skip_group_check=True)

                # Wk^T tiles: wkT[p, ai, d'] = Wk[d', ai*128+p]
                wkraw = keyq_p.tile([128, 4, A], f32, tag="kq", name="kq")
                nc.sync.dma_start(wkraw[:], wk_d.rearrange("(dc p) a -> p dc a", p=128))
                wkT = keyT_p.tile([128, 4, D], f32, tag="kT", name="kT")
                for dc in range(4):
                    pw = psT.tile([128, 512], f32, tag="pT", name="pT")
                    for ai in range(4):
                        nc.tensor.transpose(
                            pw[:, ai * 128:(ai + 1) * 128],
                            wkraw[:, dc, ai * 128:(ai + 1) * 128], ident[:, :])
                    for ai in range(4):
                        nc.vector.tensor_copy(
                            wkT[:, ai, dc * 128:(dc + 1) * 128],
                            pw[:, ai * 128:(ai + 1) * 128])

                # wT[b, d'] = sum_a qm'[b, a] Wk[d', a]
                pwT = psT.tile([B, D], f32, tag="pT", name="pT")
                for ai in range(4):
                    nc.tensor.matmul(pwT[:], qmT[:, ai, :], wkT[:, ai, :],
                                     start=(ai == 0), stop=(ai == 3))
                wT = prol.tile([B, D], f32, tag="wT")
                nc.vector.tensor_copy(wT[:], pwT[:])
                for dc in range(4):
                    pt = psT.tile([128, B], f32, tag="pT", name="pT")
                    nc.tensor.transpose(pt[:], wT[:, dc * 128:(dc + 1) * 128],
                                        ident[0:B, 0:B])
                    nc.vector.tensor_copy(w2all[:, dc, t, :], pt[:])

            # add r to the mono column via ones x r outer product
            nc.tensor.matmul(pc2[:, 0:1], ones1[:], r2[0:1, 0:1],
                             start=False, stop=True, skip_group_check=True)
            nc.tensor.matmul(pc2[:, 1:2], ones1[:], r2[0:1, 1:2],
                             start=False, stop=True, skip_group_check=True)
            nc.vector.tensor_copy(c2T[:], pc2[:])
            nc.gpsimd.dma_start(cg0[:], c2T[0:GB, :])
            nc.gpsimd.dma_start(cg1[:], c2T[GB:B, :])
        c_g = [cg0, cg1]

        def phase1_batch(b):
            g, j = divmod(b, GB)
            st_e = ste_p.tile([2, K], f32, tag="st_e", name="st_e")
            for q in range(NQ):
                k0 = q * 512
                nk = 512 if q < NQ - 1 else KL
                kq = keyq_p.tile([128, 4, 512], f32, tag="kq", name="kq")
                if q < NQ - 1:
                    nc.sync.dma_start(
                        kq[:], key_d[b, k0:k0 + 512, :]
                        .rearrange("(ks p) d -> p ks d", p=128))
                else:
                    nc.gpsimd.memset(kq[64:128, 3, :], 0.0)
                    nc.sync.dma_start(
                        kq[:, 0:3, :], key_d[b, k0:k0 + 384, :]
                        .rearrange("(ks p) d -> p ks d", p=128))
                    nc.sync.dma_start(kq[0:80, 3, :], key_d[b, k0 + 384:K, :])

                kT = keyT_p.tile([128, 4, 4, 128], f32r, tag="kT", name="kT")
                for dj in range(4):
                    pT = psT.tile([128, 512], f32, tag="pT", name="pT")
                    for ks in range(4):
                        nc.tensor.transpose(
                            pT[:, ks * 128:(ks + 1) * 128],
                            kq[:, ks, dj * 128:(dj + 1) * 128], ident[:, :])
                    nc.scalar.copy(kT[:, dj, :, :],
                                   pT[:].rearrange("p (ks kk) -> p ks kk", ks=4))

                pE = psE.tile([2, 512], f32, tag="pE", name="pE")
                for di in range(4):
                    nc.tensor.matmul(pE[:], w2all[:, di, :, b], kT[:, di, :, :],
                                     start=(di == 0), stop=(di == 3))
                nc.scalar.copy(st_e[:, k0:k0 + nk], pE[:, 0:nk])
            nc.gpsimd.dma_start(e_mono_g[g][j:j + 1, :], st_e[0:1, :])
            nc.gpsimd.dma_start(e_chunk_g[g][j:j + 1, :], st_e[1:2, :])

        def sp():
            return scan_p.tile([GB, K], f32, tag="s2k", name="s2k", bufs=4)

        def phase2_group(g):
            b0 = g * GB
            mski = scan_p.tile([GB, K], i32, tag="s2k", name="s2k", bufs=4)
            nc.gpsimd.dma_start(mski[:],
                                msk_d[b0:b0 + GB].rearrange("b one k -> b (one k)"))
            emm = sp()
            nc.vector.memset(emm[:], NEG)
            nc.vector.copy_predicated(emm[:], mski[:], e_mono_g[g][:])
            ecm = sp()
            nc.vector.memset(ecm[:], NEG)
            nc.vector.copy_predicated(ecm[:], mski[:], e_chunk_g[g][:])

            # chunk side (c_chunk shift cancels in beta's normalization)
            negmax = scan_p.tile([GB, 1], f32, tag="negmax")
            nc.vector.tensor_reduce(negmax[:], ecm[:], mybir.AxisListType.X,
                                    OP.max, negate=True)
            sm = sp()
            nc.scalar.activation(sm[:], ecm[:], AF.Exp, bias=negmax[:])
            smc = scan_p.tile([GB, K], f32, tag="smc")
            nc.vector.tensor_scalar(smc[:], sm[:], 1e-5, None, OP.max)
            d1 = sp()
            nc.vector.tensor_tensor(d1[:, 1:], smc[:, 1:], smc[:, :-1], OP.add)
            nc.vector.tensor_copy(d1[:, 0:1], smc[:, 0:1])
            d2 = sp()
            nc.vector.tensor_tensor(d2[:, 2:], d1[:, 2:], d1[:, :-2], OP.add)
            nc.vector.tensor_copy(d2[:, 0:2], d1[:, 0:2])
            den = sp()
            nc.vector.tensor_tensor(den[:, 4:], d2[:, 4:], d2[:, :-4], OP.add)
            nc.vector.tensor_copy(den[:, 0:4], d2[:, 0:4])
            rden = scan_p.tile([GB, K], f32, tag="rden")
            nc.vector.reciprocal(rden[:], den[:])

            # mono side
            noise_g = sp()
            nc.gpsimd.dma_start(noise_g[:], nz_d[b0:b0 + GB, :])
            x = sp()
            nc.vector.scalar_tensor_tensor(x[:], emm[:], c_g[g][:, 0:1],
                                           noise_g[:], OP.add, OP.add)
            p = scan_p.tile([GB, K], f32, tag="p")
            nc.scalar.activation(p[:], x[:], AF.Sigmoid)
            q1 = sp()
            nc.vector.tensor_scalar(q1[:], p[:], -1.0, 1.0, OP.mult, OP.add)
            q1c = sp()
            nc.vector.tensor_scalar(q1c[:], q1[:], EPS, 1.0, OP.max, OP.min)
            cprod = scan_p.tile([GB, K], f32, tag="cprod")
            nc.vector.memset(cprod[:, 0:1], 1.0)
            nc.vector.tensor_tensor_scan(cprod[:, 1:], q1c[:, :K - 1],
                                         q1c[:, :K - 1], 1.0, OP.mult, OP.bypass)
            cpc = sp()
            nc.vector.tensor_scalar(cpc[:], cprod[:], EPS, 1.0, OP.max, OP.min)
            rcp = sp()
            nc.vector.reciprocal(rcp[:], cpc[:])
            aw_g = sp()
            nc.gpsimd.dma_start(aw_g[:], aw_d[b0:b0 + GB, :])
            taw = sp()
            nc.vector.tensor_tensor(taw[:], aw_g[:], rcp[:], OP.mult)
            csum = sp()
            nc.vector.tensor_tensor_scan(csum[:], taw[:], taw[:], 0.0,
                                         OP.add, OP.bypass)
            al1 = sp()
            nc.vector.tensor_tensor(al1[:], p[:], cprod[:], OP.mult)
            alpha = sp()
            nc.vector.tensor_tensor(alpha[:], al1[:], csum[:], OP.mult)
            nc.gpsimd.dma_start(al_d[b0:b0 + GB, :], alpha[:])

            u = sp()
            nc.vector.tensor_tensor(u[:], alpha[:], rden[:], OP.mult)
            f1 = sp()
            nc.vector.tensor_tensor(f1[:, :-1], u[:, :-1], u[:, 1:], OP.add)
            nc.vector.tensor_copy(f1[:, K - 1:], u[:, K - 1:])
            f2 = sp()
            nc.vector.tensor_tensor(f2[:, :-2], f1[:, :-2], f1[:, 2:], OP.add)
            nc.vector.tensor_copy(f2[:, K - 2:], f1[:, K - 2:])
            ms2 = sp()
            nc.vector.tensor_tensor(ms2[:, :-4], f2[:, :-4], f2[:, 4:], OP.add)
            nc.vector.tensor_copy(ms2[:, K - 4:], f2[:, K - 4:])
            beta = scan_p.tile([GB, K], f32, tag="beta")
            nc.vector.tensor_tensor(beta[:], smc[:], ms2[:], OP.mult)

            for kt in range(16):
                k0 = kt * 128
                nk = min(128, K - k0)
                pB = psT.tile([128, 512], f32, tag="pT", name="pT")
                if nk < 128:
                    nc.vector.memset(pB[64:128, 0:GB], 0.0)
                nc.tensor.transpose(pB[0:nk, 0:GB], beta[:, k0:k0 + nk],
                                    ident[0:GB, 0:GB])
                nc.vector.tensor_copy(betaT_g[g][:, kt, :], pB[:, 0:GB])

        def phase3_group(g):
            for j in range(GB):
                b = g * GB + j
                pcv = psC.tile([1, 512], f32, tag="pcv", name="pcv")
                for q in range(NQ):
                    k0 = q * 512
                    vq = valr_p.tile([128, 4, 512], f32r, tag="vq", name="vq")
                    if q < NQ - 1:
                        nc.gpsimd.dma_start(
                            vq[:], val_d[b, k0:k0 + 512, :]
                            .rearrange("(ks p) d -> p ks d", p=128))
                    else:
                        nc.gpsimd.memset(vq[64:128, 3, :].bitcast(f32), 0.0)
                        nc.gpsimd.dma_start(
                            vq[:, 0:3, :], val_d[b, k0:k0 + 384, :]
                            .rearrange("(ks p) d -> p ks d", p=128))
                        nc.gpsimd.dma_start(vq[0:80, 3, :], val_d[b, k0 + 384:K, :])
                    for ks in range(4):
                        kt = q * 4 + ks
                        nc.tensor.matmul(pcv[:], betaT_g[g][:, kt, j:j + 1],
                                         vq[:, ks, :],
                                         start=(kt == 0), stop=(kt == 15))
                cvrow = ste_p.tile([1, D], f32, tag="cvrow", name="cvrow")
                nc.scalar.copy(cvrow[:], pcv[:])
                nc.gpsimd.dma_start(cv_d[b, :, :], cvrow[:])

        loop_cm = tc.For_i(0, reps, 1) if reps > 1 else nullcontext()
        with loop_cm:
            for b in range(4):
                phase1_batch(b)
            phase2_group(0)
            for b in range(4, B):
                phase1_batch(b)
            phase3_group(0)
            phase2_group(1)
            phase3_group(1)

    nc.compile()
    return nc


_NC_CACHE = None


def kernel(**inputs):
    global _NC_CACHE
    if _NC_CACHE is None:
        _NC_CACHE = _build_nc()
    nc = _NC_CACHE

    full_B = inputs["key"].shape[0]
    per = full_B // NCORES
    assert per == B

    shard_names = ["key", "value", "query", "mask", "aw_prev", "noise"]
    rep_names = ["Wk_mono", "bk_mono", "Wq_mono", "bq_mono", "r",
                 "Wk_chunk", "bk_chunk", "Wq_chunk", "bq_chunk"]
    in_maps = []
    for c in range(NCORES):
        m = {}
        for n in shard_names:
            m[n] = np.ascontiguousarray(
                np.asarray(inputs[n])[c * per:(c + 1) * per])
        for n in rep_names:
            m[n] = np.ascontiguousarray(np.asarray(inputs[n]))
        in_maps.append(m)

    res = run_bass_kernel_spmd(nc, in_maps, list(range(NCORES))).results
    cv = np.concatenate([r["out_cv"] for r in res], axis=0)
    alpha = np.concatenate([r["out_alpha"] for r in res], axis=0)
    return cv, alpha


# revision 27
# speedup vs baseline: 1.2083x; 1.2083x over previous
"""MoChA (monotonic chunkwise attention) Bass kernel for TRN2, 8 NeuronCores.

Sharding: data-parallel over batch B=64 -> 8 cores x 8 batches. Weights
replicated. Returns (cv [64,1,512], alpha [64,2000]) like the reference.

Algebraic reformulation (exact): with qm' = (query@Wq + bq)/sqrt(A),
w = Wk @ qm', c = qm'.bk (+r), the energies are e[k] = key[k].w + c, so the
two [2000,512]x[512,512] matmuls collapse to one [2000,512]x[512,2] per
batch. c_chunk is dropped: a per-row constant shift cancels in the
softmax/moving-sum normalization of beta.

Schedule: batches are processed in two groups of 4 so the scan phase of
group 0 overlaps the key pass of group 1, and the value pass of group 0
overlaps the scan phase of group 1 -- keeps the DMA ring busy end-to-end.
"""
import sys
for _p in ("/opt/trn_rl_repo", "/opt/pypackages"):
    if _p not in sys.path:
        sys.path.append(_p)

import numpy as np
from contextlib import ExitStack, nullcontext

import concourse.bass as bass
import concourse.bacc as bacc
import concourse.tile as tile
from concourse import mybir, masks
from concourse.bass_utils import run_bass_kernel_spmd

f32 = mybir.dt.float32
f32r = mybir.dt.float32r
i32 = mybir.dt.int32
AF = mybir.ActivationFunctionType
OP = mybir.AluOpType
MS = bass.MemorySpace

NCORES = 8
B = 8               # batches per core
GB = 4              # batches per group (2 groups)
K = 2000
D = 512
A = 512
NEG = float(np.finfo(np.float32).min)
EPS = 1e-6
SCALE = float(np.sqrt(A))
NQ = 4              # quarters of K, 512 k's each (last has 464)
KL = K - 3 * 512    # 464


def _build_nc(reps=1):
    nc = bacc.Bacc("TRN2", target_bir_lowering=False, debug=False)

    key_d = nc.dram_tensor("key", [B, K, D], f32, kind="ExternalInput").ap()
    val_d = nc.dram_tensor("value", [B, K, D], f32, kind="ExternalInput").ap()
    qry_d = nc.dram_tensor("query", [B, 1, D], f32, kind="ExternalInput").ap()
    msk_d = nc.dram_tensor("mask", [B, 1, K], i32, kind="ExternalInput").ap()
    aw_d = nc.dram_tensor("aw_prev", [B, K], f32, kind="ExternalInput").ap()
    nz_d = nc.dram_tensor("noise", [B, K], f32, kind="ExternalInput").ap()
    wkm_d = nc.dram_tensor("Wk_mono", [D, A], f32, kind="ExternalInput").ap()
    bkm_d = nc.dram_tensor("bk_mono", [A], f32, kind="ExternalInput").ap()
    wqm_d = nc.dram_tensor("Wq_mono", [D, A], f32, kind="ExternalInput").ap()
    bqm_d = nc.dram_tensor("bq_mono", [A], f32, kind="ExternalInput").ap()
    r_d = nc.dram_tensor("r", [1], f32, kind="ExternalInput").ap()
    wkc_d = nc.dram_tensor("Wk_chunk", [D, A], f32, kind="ExternalInput").ap()
    bkc_d = nc.dram_tensor("bk_chunk", [A], f32, kind="ExternalInput").ap()
    wqc_d = nc.dram_tensor("Wq_chunk", [D, A], f32, kind="ExternalInput").ap()
    bqc_d = nc.dram_tensor("bq_chunk", [A], f32, kind="ExternalInput").ap()

    cv_d = nc.dram_tensor("out_cv", [B, 1, D], f32, kind="ExternalOutput").ap()
    al_d = nc.dram_tensor("out_alpha", [B, K], f32, kind="ExternalOutput").ap()

    with tile.TileContext(nc) as tc, ExitStack() as ctx:
        const = ctx.enter_context(tc.tile_pool(name="const", bufs=1))
        keyq_p = ctx.enter_context(tc.tile_pool(name="keyq", bufs=3))
        keyT_p = ctx.enter_context(tc.tile_pool(name="keyT", bufs=2))
        valr_p = ctx.enter_context(tc.tile_pool(name="valr", bufs=2))
        scan_p = ctx.enter_context(tc.tile_pool(name="scan", bufs=1))
        ste_p = ctx.enter_context(tc.tile_pool(name="ste", bufs=2))
        psT = ctx.enter_context(tc.tile_pool(name="psT", bufs=3, space=MS.PSUM))
        psE = ctx.enter_context(tc.tile_pool(name="psE", bufs=2, space=MS.PSUM))
        psC = ctx.enter_context(tc.tile_pool(name="psC", bufs=2, space=MS.PSUM))

        ident = const.tile([128, 128], f32)
        masks.make_identity(nc, ident[:])
        ones1 = const.tile([1, B], f32)
        nc.gpsimd.memset(ones1[:], 1.0)

        w2all = const.tile([128, 4, 2, B], f32r)     # [d_in_chunk, dchunk, type, b]
        c2T = const.tile([B, 2], f32)               # per-batch bias (mono incl r, -)
        cg0 = const.tile([GB, 2], f32)
        cg1 = const.tile([GB, 2], f32)
        bT0 = const.tile([128, 16, GB], f32r)
        bT1 = const.tile([128, 16, GB], f32r)
        betaT_g = [bT0, bT1]
        em0_t = const.tile([GB, K], f32)
        em1_t = const.tile([GB, K], f32)
        ec0_t = const.tile([GB, K], f32)
        ec1_t = const.tile([GB, K], f32)
        e_mono_g = [em0_t, em1_t]
        e_chunk_g = [ec0_t, ec1_t]

        # ---------------- prologue: per-batch w and c ----------------
        with tc.tile_pool(name="prol", bufs=1) as prol:
            q_sb = prol.tile([B, D], f32)
            nc.sync.dma_start(q_sb[:], qry_d.rearrange("b one d -> b (one d)"))
            r2 = prol.tile([1, 2], f32)
            nc.gpsimd.memset(r2[:], 0.0)
            nc.gpsimd.dma_start(r2[0:1, 0:1], r_d.rearrange("(one x) -> one x", one=1))
            bk2 = prol.tile([128, 4, 2], f32)
            nc.gpsimd.dma_start(bk2[:, :, 0], bkm_d.rearrange("(ai p) -> p ai", p=128))
            nc.gpsimd.dma_start(bk2[:, :, 1], bkc_d.rearrange("(ai p) -> p ai", p=128))

            qT = prol.tile([128, 4, B], f32)
            for dc in range(4):
                pt = psT.tile([128, B], f32, tag="pT", name="pT")
                nc.tensor.transpose(pt[:], q_sb[:, dc * 128:(dc + 1) * 128],
                                    ident[0:B, 0:B])
                nc.vector.tensor_copy(qT[:, dc, :], pt[:])

            pc2 = psC.tile([B, 2], f32, tag="pcv", name="pcv")
            for t, (wq_d, bq_d, wk_d) in enumerate(
                    [(wqm_d, bqm_d, wkm_d), (wqc_d, bqc_d, wkc_d)]):
                wq = keyq_p.tile([128, 4, A], f32, tag="kq", name="kq")
                nc.sync.dma_start(wq[:], wq_d.rearrange("(dc p) a -> p dc a", p=128))
                bq = prol.tile([1, A], f32, tag="bq")
                nc.sync.dma_start(bq[:], bq_d.rearrange("(one a) -> one a", one=1))

                # qm' = (q @ Wq + bq) / SCALE        [B, A]
                pqm = psT.tile([B, A], f32, tag="pT", name="pT")
                for dc in range(4):
                    nc.tensor.matmul(pqm[:], qT[:, dc, :], wq[:, dc, :],
                                     start=(dc == 0), stop=False)
                nc.tensor.matmul(pqm[:], ones1[:], bq[:], start=False, stop=True)
                qm = prol.tile([B, A], f32, tag="qm")
                nc.scalar.activation(qm[:], pqm[:], AF.Copy, scale=1.0 / SCALE)

                qmT = prol.tile([128, 4, B], f32, tag="qmT")
                for ai in range(4):
                    pt = psT.tile([128, B], f32, tag="pT", name="pT")
                    nc.tensor.transpose(pt[:], qm[:, ai * 128:(ai + 1) * 128],
                                        ident[0:B, 0:B])
                    nc.vector.tensor_copy(qmT[:, ai, :], pt[:])

                # c2T[:, t] = qm' . bk_t  (accumulated across both types)
                for ai in range(4):
                    nc.tensor.matmul(pc2[:, t:t + 1], qmT[:, ai, :],
                                     bk2[:, ai, t:t + 1],
                                     start=(ai == 0), stop=False,
                                     skip_group_check=True)

                # Wk^T tiles: wkT[p, ai, d'] = Wk[d', ai*128+p]
                wkraw = keyq_p.tile([128, 4, A], f32, tag="kq", name="kq")
                nc.sync.dma_start(wkraw[:], wk_d.rearrange("(dc p) a -> p dc a", p=128))
                wkT = keyT_p.tile([128, 4, D], f32, tag="kT", name="kT")
                for dc in range(4):
                    pw = psT.tile([128, 512], f32, tag="pT", name="pT")
                    for ai in range(4):
                        nc.tensor.transpose(
                            pw[:, ai * 128:(ai + 1) * 128],
                            wkraw[:, dc, ai * 128:(ai + 1) * 128], ident[:, :])
                    for ai in range(4):
                        nc.vector.tensor_copy(
                            wkT[:, ai, dc * 128:(dc + 1) * 128],
                            pw[:, ai * 128:(ai + 1) * 128])

                # wT[b, d'] = sum_a qm'[b, a] Wk[d', a]
                pwT = psT.tile([B, D], f32, tag="pT", name="pT")
                for ai in range(4):
                    nc.tensor.matmul(pwT[:], qmT[:, ai, :], wkT[:, ai, :],
                                     start=(ai == 0), stop=(ai == 3))
                wT = prol.tile([B, D], f32, tag="wT")
                nc.vector.tensor_copy(wT[:], pwT[:])
                for dc in range(4):
                    pt = psT.tile([128, B], f32, tag="pT", name="pT")
                    nc.tensor.transpose(pt[:], wT[:, dc * 128:(dc + 1) * 128],
                                        ident[0:B, 0:B])
                    nc.vector.tensor_copy(w2all[:, dc, t, :], pt[:])

            # add r to the mono column via ones x r outer product
            nc.tensor.matmul(pc2[:, 0:1], ones1[:], r2[0:1, 0:1],
                             start=False, stop=True, skip_group_check=True)
            nc.tensor.matmul(pc2[:, 1:2], ones1[:], r2[0:1, 1:2],
                             start=False, stop=True, skip_group_check=True)
            nc.vector.tensor_copy(c2T[:], pc2[:])
            nc.gpsimd.dma_start(cg0[:], c2T[0:GB, :])
            nc.gpsimd.dma_start(cg1[:], c2T[GB:B, :])
        c_g = [cg0, cg1]

        def phase1_batch(b):
            g, j = divmod(b, GB)
            st_e = ste_p.tile([2, K], f32, tag="st_e", name="st_e")
            for q in range(NQ):
                k0 = q * 512
                nk = 512 if q < NQ - 1 else KL
                kq = keyq_p.tile([128, 4, 512], f32, tag="kq", name="kq")
                if q < NQ - 1:
                    nc.sync.dma_start(
                        kq[:], key_d[b, k0:k0 + 512, :]
                        .rearrange("(ks p) d -> p ks d", p=128))
                else:
                    nc.sync.dma_start(
                        kq[:, 0:3, :], key_d[b, k0:k0 + 384, :]
                        .rearrange("(ks p) d -> p ks d", p=128))
                    nc.sync.dma_start(kq[0:80, 3, :], key_d[b, k0 + 384:K, :])

                kT = keyT_p.tile([128, 4, 4, 128], f32r, tag="kT", name="kT")
                for dj in range(4):
                    pT = psT.tile([128, 512], f32, tag="pT", name="pT")
                    for ks in range(4):
                        nc.tensor.transpose(
                            pT[:, ks * 128:(ks + 1) * 128],
                            kq[:, ks, dj * 128:(dj + 1) * 128], ident[:, :])
                    nc.scalar.copy(kT[:, dj, :, :],
                                   pT[:].rearrange("p (ks kk) -> p ks kk", ks=4))

                pE = psE.tile([2, 512], f32, tag="pE", name="pE")
                for di in range(4):
                    nc.tensor.matmul(pE[:], w2all[:, di, :, b], kT[:, di, :, :],
                                     start=(di == 0), stop=(di == 3))
                nc.scalar.copy(st_e[:, k0:k0 + nk], pE[:, 0:nk])
            nc.gpsimd.dma_start(e_mono_g[g][j:j + 1, :], st_e[0:1, :])
            nc.gpsimd.dma_start(e_chunk_g[g][j:j + 1, :], st_e[1:2, :])

        def sp():
            return scan_p.tile([GB, K], f32, tag="s2k", name="s2k", bufs=4)

        def phase2_group(g):
            b0 = g * GB
            mski = scan_p.tile([GB, K], i32, tag="s2k", name="s2k", bufs=4)
            nc.gpsimd.dma_start(mski[:],
                                msk_d[b0:b0 + GB].rearrange("b one k -> b (one k)"))
            emm = sp()
            nc.vector.memset(emm[:], NEG)
            nc.vector.copy_predicated(emm[:], mski[:], e_mono_g[g][:])
            ecm = sp()
            nc.vector.memset(ecm[:], NEG)
            nc.vector.copy_predicated(ecm[:], mski[:], e_chunk_g[g][:])

            # chunk side (c_chunk shift cancels in beta's normalization)
            negmax = scan_p.tile([GB, 1], f32, tag="negmax")
            nc.vector.tensor_reduce(negmax[:], ecm[:], mybir.AxisListType.X,
                                    OP.max, negate=True)
            sm = sp()
            nc.scalar.activation(sm[:], ecm[:], AF.Exp, bias=negmax[:])
            smc = scan_p.tile([GB, K], f32, tag="smc")
            nc.vector.tensor_scalar(smc[:], sm[:], 1e-5, None, OP.max)
            d1 = sp()
            nc.vector.tensor_tensor(d1[:, 1:], smc[:, 1:], smc[:, :-1], OP.add)
            nc.vector.tensor_copy(d1[:, 0:1], smc[:, 0:1])
            d2 = sp()
            nc.vector.tensor_tensor(d2[:, 2:], d1[:, 2:], d1[:, :-2], OP.add)
            nc.vector.tensor_copy(d2[:, 0:2], d1[:, 0:2])
            den = sp()
            nc.vector.tensor_tensor(den[:, 4:], d2[:, 4:], d2[:, :-4], OP.add)
            nc.vector.tensor_copy(den[:, 0:4], d2[:, 0:4])
            rden = scan_p.tile([GB, K], f32, tag="rden")
            nc.vector.reciprocal(rden[:], den[:])

            # mono side
            noise_g = sp()
            nc.gpsimd.dma_start(noise_g[:], nz_d[b0:b0 + GB, :])
            x = sp()
            nc.vector.scalar_tensor_tensor(x[:], emm[:], c_g[g][:, 0:1],
                                           noise_g[:], OP.add, OP.add)
            p = scan_p.tile([GB, K], f32, tag="p")
            nc.scalar.activation(p[:], x[:], AF.Sigmoid)
            q1 = sp()
            nc.vector.tensor_scalar(q1[:], p[:], -1.0, 1.0, OP.mult, OP.add)
            q1c = sp()
            nc.vector.tensor_scalar(q1c[:], q1[:], EPS, 1.0, OP.max, OP.min)
            cprod = scan_p.tile([GB, K], f32, tag="cprod")
            nc.vector.memset(cprod[:, 0:1], 1.0)
            nc.vector.tensor_tensor_scan(cprod[:, 1:], q1c[:, :K - 1],
                                         q1c[:, :K - 1], 1.0, OP.mult, OP.bypass)
            cpc = sp()
            nc.vector.tensor_scalar(cpc[:], cprod[:], EPS, 1.0, OP.max, OP.min)
            rcp = sp()
            nc.vector.reciprocal(rcp[:], cpc[:])
            aw_g = sp()
            nc.gpsimd.dma_start(aw_g[:], aw_d[b0:b0 + GB, :])
            taw = sp()
            nc.vector.tensor_tensor(taw[:], aw_g[:], rcp[:], OP.mult)
            csum = sp()
            nc.vector.tensor_tensor_scan(csum[:], taw[:], taw[:], 0.0,
                                         OP.add, OP.bypass)
            al1 = sp()
            nc.vector.tensor_tensor(al1[:], p[:], cprod[:], OP.mult)
            alpha = sp()
            nc.vector.tensor_tensor(alpha[:], al1[:], csum[:], OP.mult)
            nc.gpsimd.dma_start(al_d[b0:b0 + GB, :], alpha[:])

            u = sp()
            nc.vector.tensor_tensor(u[:], alpha[:], rden[:], OP.mult)
            f1 = sp()
            nc.vector.tensor_tensor(f1[:, :-1], u[:, :-1], u[:, 1:], OP.add)
            nc.vector.tensor_copy(f1[:, K - 1:], u[:, K - 1:])
            f2 = sp()
            nc.vector.tensor_tensor(f2[:, :-2], f1[:, :-2], f1[:, 2:], OP.add)
            nc.vector.tensor_copy(f2[:, K - 2:], f1[:, K - 2:])
            ms2 = sp()
            nc.vector.tensor_tensor(ms2[:, :-4], f2[:, :-4], f2[:, 4:], OP.add)
            nc.vector.tensor_copy(ms2[:, K - 4:], f2[:, K - 4:])
            beta = scan_p.tile([GB, K], f32, tag="beta")
            nc.vector.tensor_tensor(beta[:], smc[:], ms2[:], OP.mult)

            for kt in range(16):
                k0 = kt * 128
                nk = min(128, K - k0)
                pB = psT.tile([128, 512], f32, tag="pT", name="pT")
                if nk < 128:
                    nc.vector.memset(pB[64:128, 0:GB], 0.0)
                nc.tensor.transpose(pB[0:nk, 0:GB], beta[:, k0:k0 + nk],
                                    ident[0:GB, 0:GB])
                nc.vector.tensor_copy(betaT_g[g][:, kt, :], pB[:, 0:GB])

        def phase3_group(g):
            for j in range(GB):
                b = g * GB + j
                pcv = psC.tile([1, 512], f32, tag="pcv", name="pcv")
                for q in range(NQ):
                    k0 = q * 512
                    vqr = keyq_p.tile([128, 4, 512], f32, tag="kq", name="kq")
                    if q < NQ - 1:
                        nc.sync.dma_start(
                            vqr[:], val_d[b, k0:k0 + 512, :]
                            .rearrange("(ks p) d -> p ks d", p=128))
                    else:
                        nc.sync.dma_start(
                            vqr[:, 0:3, :], val_d[b, k0:k0 + 384, :]
                            .rearrange("(ks p) d -> p ks d", p=128))
                        nc.sync.dma_start(vqr[0:80, 3, :], val_d[b, k0 + 384:K, :])
                    vq = valr_p.tile([128, 4, 512], f32r, tag="vq", name="vq")
                    nc.vector.tensor_copy(vq[:], vqr[:])
                    for ks in range(4):
                        kt = q * 4 + ks
                        nc.tensor.matmul(pcv[:], betaT_g[g][:, kt, j:j + 1],
                                         vq[:, ks, :],
                                         start=(kt == 0), stop=(kt == 15))
                cvrow = ste_p.tile([1, D], f32, tag="cvrow", name="cvrow")
                nc.scalar.copy(cvrow[:], pcv[:])
                nc.gpsimd.dma_start(cv_d[b, :, :], cvrow[:])

        loop_cm = tc.For_i(0, reps, 1) if reps > 1 else nullcontext()
        with loop_cm:
            for b in range(5):
                phase1_batch(b)
            phase2_group(0)
            for b in range(5, B):
                phase1_batch(b)
            phase3_group(0)
            phase2_group(1)
            phase3_group(1)

    nc.compile()
    return nc


_NC_CACHE = None


def kernel(**inputs):
    global _NC_CACHE
    if _NC_CACHE is None:
        _NC_CACHE = _build_nc()
    nc = _NC_CACHE

    full_B = inputs["key"].shape[0]
    per = full_B // NCORES
    assert per == B

    shard_names = ["key", "value", "query", "mask", "aw_prev", "noise"]
    rep_names = ["Wk_mono", "bk_mono", "Wq_mono", "bq_mono", "r",
                 "Wk_chunk", "bk_chunk", "Wq_chunk", "bq_chunk"]
    in_maps = []
    for c in range(NCORES):
        m = {}
        for n in shard_names:
            m[n] = np.ascontiguousarray(
                np.asarray(inputs[n])[c * per:(c + 1) * per])
        for n in rep_names:
            m[n] = np.ascontiguousarray(np.asarray(inputs[n]))
        in_maps.append(m)

    res = run_bass_kernel_spmd(nc, in_maps, list(range(NCORES))).results
    cv = np.concatenate([r["out_cv"] for r in res], axis=0)
    alpha = np.concatenate([r["out_alpha"] for r in res], axis=0)
    return cv, alpha
